# revision 15
# baseline (speedup 1.0000x reference)
"""Trainium2 Bass kernel for MinibatchDiscrimination.

Reference computation (fp32):
    m = (x @ W.T + b).reshape(nb, 64, 16)            # nb=512
    d[i,j,B] = sum_c |m[i,B,c] - m[j,B,c]|
    o[i,B]   = sum_j exp(-d[i,j,B])
    out      = concat(x, o, axis=1)                   # (512, 1088)

Strategy (8 cores, data-parallel over output rows i):
  Each core gets x row-rotated so its 64 rows are local rows 0..63, with
  x pre-transposed on host (xT [1024,512]) plus W^T so no device transposes
  are needed.  On device:
    mT[t] = W @ x^T   as 8 tiles [128 (B,c), 512 j]          (PE)
  Pairwise, using |a-b| = a + b - 2*min(a,b) on half the tiles so the work
  splits across ScalarE (Abs path) and VectorE (min path):
    t <  A: absT = Abs(m_i - mT[t])  (ACT, scale=-1, bias=m_i col)
            psum -= csum_c absT      (PE matmul, indicator = -1)
    t >= A: minT = min(mT[t], m_i)   (DVE scalar_tensor_tensor)
            psum += 2*csum_c minT    (PE matmul, indicator = +2)
  With S47 = sum_c m over the min-path features:
    exp(-d) = exp(psum - S47_i) * exp(-S47_j)
  Q = exp(-S47) [64,512] is i-independent (computed once);
    E = Exp(psum + bias=-S47_i)      (ACT)
    o[:,i] = sum_j E*Q               (DVE scalar_tensor_tensor accum_out)
Host assembles: out = concat(x, gather(o_core.T), axis=1).
"""

import sys
import numpy as np

if "/opt/trn_rl_repo" not in sys.path:
    sys.path.insert(0, "/opt/trn_rl_repo")

NB = 512          # batch rows
NIN = 1024        # n_in
NBF = 64          # n_B
NCD = 16          # n_C
FOUT = NBF * NCD  # 1024 projection features
NCORES = 8
IB = NB // NCORES  # 64 output rows per core
A_SPLIT = 3        # feature tiles [0,A) -> ACT abs path; [A,8) -> DVE min path

_CACHE = {}


def _build_program():
    import concourse.bass as bass
    import concourse.tile as tile
    from concourse import mybir
    from contextlib import ExitStack

    f32 = mybir.dt.float32
    Alu = mybir.AluOpType
    Act = mybir.ActivationFunctionType

    nc = bass.Bass()
    xT_d = nc.declare_dram_parameter("xT", [NIN, NB], f32, isOutput=False)
    wT_d = nc.declare_dram_parameter("wT", [NIN, FOUT], f32, isOutput=False)
    b_d = nc.declare_dram_parameter("b", [FOUT], f32, isOutput=False)
    indA_d = nc.declare_dram_parameter("indA", [FOUT, NBF], f32, isOutput=False)
    indM_d = nc.declare_dram_parameter("indM", [FOUT, NBF], f32, isOutput=False)
    o_d = nc.declare_dram_parameter("o", [NBF, IB], f32, isOutput=True)

    with tile.TileContext(nc) as tc, ExitStack() as ctx:
        singles = ctx.enter_context(tc.tile_pool(name="singles", bufs=1))
        wstream = ctx.enter_context(tc.tile_pool(name="wstream", bufs=6))
        scratch = ctx.enter_context(tc.tile_pool(name="scratch", bufs=4))
        epool = ctx.enter_context(tc.tile_pool(name="epool", bufs=3))
        psA = ctx.enter_context(tc.tile_pool(name="psA", bufs=2, space="PSUM"))
        psB = ctx.enter_context(tc.tile_pool(name="psB", bufs=3, space="PSUM"))
        psQ = ctx.enter_context(tc.tile_pool(name="psQ", bufs=1, space="PSUM"))

        dma = nc.default_dma_engine

        # ---- persistent loads -------------------------------------------
        xT_sb = []
        for k in range(8):
            t_ = singles.tile([128, NB], f32, name=f"xT{k}", tag=f"xT{k}")
            dma.dma_start(out=t_, in_=xT_d[128 * k : 128 * (k + 1), :])
            xT_sb.append(t_)

        b_sb = singles.tile([128, 8], f32, name="b_sb", tag="b_sb")
        dma.dma_start(out=b_sb, in_=b_d.rearrange("(t p) -> p t", p=128))

        ind_sb = []  # per feature-tile indicator stationary [128, 64]
        for t in range(8):
            src = indA_d if t < A_SPLIT else indM_d
            t_ = singles.tile([128, NBF], f32, name=f"ind{t}", tag=f"ind{t}")
            dma.dma_start(out=t_, in_=src[128 * t : 128 * (t + 1), :])
            ind_sb.append(t_)

        # ---- mT = W @ x^T  (8 tiles [128 f, 512 j]) ---------------------
        mT = []
        for t in range(8):
            ps = psA.tile([128, NB], f32, name="mps", tag="mps")
            for kb in range(8):
                wblk = wstream.tile([128, 128], f32, name="wblk", tag="wblk")
                dma.dma_start(
                    out=wblk,
                    in_=wT_d[128 * kb : 128 * (kb + 1), 128 * t : 128 * (t + 1)],
                )
                nc.tensor.matmul(
                    ps, lhsT=wblk, rhs=xT_sb[kb],
                    start=(kb == 0), stop=(kb == 7),
                )
            mt = singles.tile([128, NB], f32, name=f"mT{t}", tag=f"mT{t}")
            nc.scalar.activation(
                out=mt, in_=ps, func=Act.Identity,
                bias=b_sb[:, t : t + 1], scale=1.0,
            )
            mT.append(mt)

        # ---- Q = exp(-S47), negS47i = -S47 over i columns ---------------
        psq = psQ.tile([NBF, NB], f32, name="psq", tag="psq")
        for t in range(A_SPLIT, 8):
            nc.tensor.matmul(
                psq, lhsT=ind_sb[t], rhs=mT[t],
                start=(t == A_SPLIT), stop=(t == 7),
            )
        # psq = 2*S47[B, j]
        negS = singles.tile([NBF, IB], f32, name="negS", tag="negS")
        nc.scalar.activation(
            out=negS, in_=psq[:, 0:IB], func=Act.Copy, bias=0.0, scale=-0.5
        )
        Q = singles.tile([NBF, NB], f32, name="Q", tag="Q")
        nc.scalar.activation(out=Q, in_=psq, func=Act.Exp, bias=0.0, scale=-0.5)

        oacc = singles.tile([NBF, IB], f32, name="oacc", tag="oacc")

        # ---- pairwise loop over the 64 local rows -----------------------
        for i in range(IB):
            psd = psB.tile([NBF, NB], f32, name="psd", tag="psd")
            for t in range(8):
                mcol = mT[t][:, i : i + 1]
                if t < A_SPLIT:
                    ab = scratch.tile([128, NB], f32, name="ab", tag="ab")
                    nc.scalar.activation(
                        out=ab, in_=mT[t], func=Act.Abs, bias=mcol, scale=-1.0
                    )
                    rhs = ab
                else:
                    mn = scratch.tile([128, NB], f32, name="mn", tag="mn")
                    nc.vector.tensor_scalar_min(mn, mT[t], mcol)
                    rhs = mn
                nc.tensor.matmul(
                    psd, lhsT=ind_sb[t], rhs=rhs,
                    start=(t == 0), stop=(t == 7),
                )
            E = epool.tile([NBF, NB], f32, name="E", tag="E")
            nc.scalar.activation(
                out=E, in_=psd, func=Act.Exp,
                bias=negS[:, i : i + 1], scale=1.0,
            )
            Escr = epool.tile([NBF, NB], f32, name="Escr", tag="Escr")
            nc.vector.scalar_tensor_tensor(
                out=Escr, in0=E, scalar=1.0, in1=Q,
                op0=Alu.mult, op1=Alu.mult,
                accum_out=oacc[:, i : i + 1],
            )

        dma.dma_start(out=o_d[:, :], in_=oacc)

    _split_multi_waits(nc, mybir)
    return nc


def _split_multi_waits(nc, mybir):
    """This container's walrus rejects any TPB instruction carrying more than
    one sync wait ("Too many sync wait commands").  Tile emits up to ~11.
    Legalize: hoist all but one wait onto single-wait NoOps inserted just
    before the instruction on the same engine queue (waits are sem-ge, so
    order is irrelevant; the queue blocks until all are satisfied)."""
    skip = ()
    f = nc.m.functions[0]
    n_split = 0
    for blk in f.blocks:
        idx = 0
        while idx < len(blk.instructions):
            inst = blk.instructions[idx]
            si = inst.sync_info
            waits = list(si.on_wait) if si is not None and si.on_wait else []
            if len(waits) > 1 and type(inst).__name__ not in skip:
                bysem = {}
                for w in waits:
                    k = w.id
                    if k not in bysem or (w.wait_value or 0) > (
                        bysem[k].wait_value or 0
                    ):
                        bysem[k] = w
                waits = list(bysem.values())
                for w in waits[:-1]:
                    nop = mybir.InstNoOp(
                        name=nc.get_next_instruction_name(), ins=[], outs=[]
                    )
                    nop.engine = inst.engine
                    nop.sync_info = mybir.SyncInfo(on_wait=[w], on_update=[])
                    blk.instructions.insert(idx, nop)
                    idx += 1
                    n_split += 1
                si.on_wait = [waits[-1]]
            idx += 1
    return n_split


def _get_program():
    if "nc" not in _CACHE:
        _CACHE["nc"] = _build_program()
    return _CACHE["nc"]


def _make_indicators():
    indA = np.zeros((FOUT, NBF), dtype=np.float32)
    indM = np.zeros((FOUT, NBF), dtype=np.float32)
    f = np.arange(FOUT)
    indA[f, f // NCD] = -1.0
    indM[f, f // NCD] = 2.0
    return indA, indM


def kernel(x, W, b):
    from concourse.bass_utils import run_bass_kernel_spmd

    x = np.ascontiguousarray(x, dtype=np.float32)
    W = np.ascontiguousarray(W, dtype=np.float32)
    b = np.ascontiguousarray(b, dtype=np.float32)

    nc = _get_program()
    indA, indM = _make_indicators()
    WT = np.ascontiguousarray(W.T)

    in_maps = []
    for c in range(NCORES):
        xr = np.roll(x, -IB * c, axis=0)
        in_maps.append({
            "xT": np.ascontiguousarray(xr.T),
            "wT": WT,
            "b": b,
            "indA": indA,
            "indM": indM,
        })

    trace = bool(int(__import__("os").environ.get("KERNEL_TRACE", "0")))
    res = run_bass_kernel_spmd(nc, in_maps, list(range(NCORES)), trace=trace)
    _CACHE["last_results"] = res

    o_full = np.empty((NB, NBF), dtype=np.float32)
    for c in range(NCORES):
        o_full[IB * c : IB * (c + 1), :] = res.results[c]["o"].T
    return np.concatenate([x, o_full], axis=1)
